# revision 1
# baseline (speedup 1.0000x reference)
"""Trainium2 Bass kernel for nn_ButterflyModule_71176198029535.

The 16 butterfly layers (paired Givens rotations with fixed angles) compose
into a single dense 256x256 linear map.  The gather (indices_in) and scatter
(idx_out) are folded into the same matrix when they are permutation-like, so
the device does one memory-bound [131072,256] x [256,256] matmul, sharded
data-parallel over the batch dim across 8 NeuronCores.

Device-side per 128-row tile:
  - DMA in natural-layout tile [128b, 256w] (1 MiB macro chunks)
  - PE transpose the two [128,128] halves into PSUM (contract dim must be on
    partitions), DVE-copy to SBUF
  - 2 accumulating matmuls against the preloaded A-matrix chunks -> PSUM
  - ACT copy PSUM->SBUF, DMA out natural layout
"""

import numpy as np

B = 131072
W = 256
LOG2W = 8
L_IN = 8
L_OUT = 8
N_CORES = 8
R = B // N_CORES          # rows per core
TILE_R = 128              # rows per compute tile
GROUP = 16                # compute tiles per DMA group (2 MiB contiguous)
N_TILES = R // TILE_R
N_GROUPS = N_TILES // GROUP

# Matmul input dtype: float32r streams fp32 at 1 cycle/row (vs 4 for float32)
# when the moving free dim is >= 256.  Verified against the fp64 reference.
USE_F32R = True

_CACHED = {}


def _strides():
    s_in = [1 << (l % LOG2W) for l in range(L_IN)]
    s_out = [1 << ((L_OUT - 1 - k) % LOG2W) for k in range(L_OUT)]
    return s_in + s_out


def _compose_matrix(angles, indices_in, idx_out, fold_scatter):
    """Return A [W, W] float32 such that out = data @ A  (when fold_scatter),
    or A' folding only the gather, with the scatter applied on the host."""
    ang = np.asarray(angles, dtype=np.float64)
    M = np.eye(W, dtype=np.float64)
    for l, s in enumerate(_strides()):
        nblk = W // (2 * s)
        blk = np.repeat(np.arange(nblk), s)
        j = np.tile(np.arange(s), nblk)
        ai = blk * 2 * s + j
        bi = blk * 2 * s + s + j
        c = np.cos(ang[l])[:, None]     # [W//2, 1] pairs in (blk, j) raveled order
        sn = np.sin(ang[l])[:, None]
        Ma = M[ai, :].copy()
        Mb = M[bi, :].copy()
        M[ai, :] = c * Ma - sn * Mb
        M[bi, :] = sn * Ma + c * Mb
    # butterfly(z) rows = z @ M.T ; gathered input z = data @ G
    G = np.zeros((W, W), dtype=np.float64)
    G[np.asarray(indices_in, dtype=np.int64), np.arange(W)] = 1.0
    A = G @ M.T
    if fold_scatter:
        S = np.zeros((W, W), dtype=np.float64)
        S[np.arange(W), np.asarray(idx_out, dtype=np.int64)] = 1.0
        A = A @ S
    return np.ascontiguousarray(A, dtype=np.float32)


def _build_bass(repeat=1):
    from contextlib import ExitStack, nullcontext

    import concourse.tile as tile
    from concourse import bacc, mybir

    f32 = mybir.dt.float32
    mm_dt = mybir.dt.float32r if USE_F32R else f32

    nc = bacc.Bacc(trn_type="TRN2", debug=False, num_devices=N_CORES)
    data = nc.dram_tensor("data", [R, W], f32, kind="ExternalInput").ap()
    amat = nc.dram_tensor("amat", [W, W], f32, kind="ExternalInput").ap()
    ident = nc.dram_tensor("ident", [TILE_R, TILE_R], f32, kind="ExternalInput").ap()
    out = nc.dram_tensor("out", [R, W], f32, kind="ExternalOutput").ap()

    # Group packing: partition p of a group tile holds GROUP consecutive DRAM
    # rows (rows g*128*GROUP + p*GROUP + j), so the 2 MiB group DMA is fully
    # contiguous in DRAM with 16 KiB-per-partition descriptor runs.  Compute
    # tile j of a group processes rows {p*GROUP + j}; the output uses the
    # identical packing so the row permutation cancels.
    data_v = data.rearrange("(g p j) w -> g p (j w)", p=TILE_R, j=GROUP)
    out_v = out.rearrange("(g p j) w -> g p (j w)", p=TILE_R, j=GROUP)

    with tile.TileContext(nc) as tc, ExitStack() as ctx:
        const_pool = ctx.enter_context(tc.tile_pool(name="const", bufs=1))
        in_pool = ctx.enter_context(tc.tile_pool(name="xin", bufs=4))
        out_pool = ctx.enter_context(tc.tile_pool(name="yout", bufs=4))
        xt_pool = ctx.enter_context(tc.tile_pool(name="xt", bufs=8))
        ps_t_pool = ctx.enter_context(tc.tile_pool(name="ps_t", bufs=4, space="PSUM"))
        ps_o_pool = ctx.enter_context(tc.tile_pool(name="ps_o", bufs=4, space="PSUM"))

        # PE (LDW/Matmult) instructions have a tiny sync-wait budget in codegen:
        # funnel every PE input through DVE so each PE op waits on one sem.
        id_stage = const_pool.tile([TILE_R, TILE_R], f32, tag="idstage")
        nc.sync.dma_start(id_stage[:], ident)
        id_sb = const_pool.tile([TILE_R, TILE_R], f32, tag="id")
        nc.vector.tensor_copy(id_sb[:], id_stage[:])
        a_sb = []
        for k in range(2):
            a_stage = const_pool.tile([TILE_R, W], f32, tag=f"astage{k}")
            nc.sync.dma_start(a_stage[:], amat[k * TILE_R:(k + 1) * TILE_R, :])
            a_k = const_pool.tile([TILE_R, W], mm_dt, tag=f"a{k}")
            # fp32r operands must be produced by a rounding instruction
            nc.vector.tensor_copy(a_k[:], a_stage[:])
            a_sb.append(a_k)

        # Warm-up PE op so the first real transpose doesn't need both a DVE
        # wait (id_sb) and a DMA wait (xin) — the fp32 LW struct carries at
        # most one sync wait.
        ps_warm = ps_t_pool.tile([TILE_R, TILE_R], f32, tag="ps_t")
        nc.tensor.transpose(ps_warm[:], id_sb[:], id_sb[:])

        rep_cm = tc.For_i(0, repeat, 1) if repeat > 1 else nullcontext()
        with rep_cm:
            _tile_loop(nc, tc, ctx, data_v, out_v, id_sb, a_sb,
                       in_pool, out_pool, xt_pool, ps_t_pool, ps_o_pool,
                       f32, mm_dt)

    nc.compile()
    return nc


def _tile_loop(nc, tc, ctx, data_v, out_v, id_sb, a_sb, in_pool, out_pool,
               xt_pool, ps_t_pool, ps_o_pool, f32, mm_dt):
    for g in range(N_GROUPS):
        xin = in_pool.tile([TILE_R, GROUP * W], f32, tag="xin")
        nc.sync.dma_start(xin[:], data_v[g])
        yout = out_pool.tile([TILE_R, GROUP * W], f32, tag="yout")
        for j in range(GROUP):
            ps_t = ps_t_pool.tile([TILE_R, W], f32, tag="ps_t")
            for k in range(2):
                nc.tensor.transpose(
                    ps_t[:, k * TILE_R:(k + 1) * TILE_R],
                    xin[:, j * W + k * TILE_R:j * W + (k + 1) * TILE_R],
                    id_sb[:],
                )
            xt = xt_pool.tile([TILE_R, W], mm_dt, tag="xt")
            nc.vector.tensor_copy(xt[:], ps_t[:])
            ps_o = ps_o_pool.tile([TILE_R, W], f32, tag="ps_o")
            for k in range(2):
                nc.tensor.matmul(
                    ps_o[:],
                    xt[:, k * TILE_R:(k + 1) * TILE_R],
                    a_sb[k][:],
                    start=(k == 0),
                    stop=(k == 1),
                )
            nc.scalar.copy(yout[:, j * W:(j + 1) * W], ps_o[:])
        nc.sync.dma_start(out_v[g], yout[:])


def _get_bass(repeat=1):
    key = ("nc", repeat)
    if key not in _CACHED:
        _CACHED[key] = _build_bass(repeat)
    return _CACHED[key]


def _run_device(data_f32, A, trace=False, trace_kwargs=None):
    from concourse.bass_utils import run_bass_kernel_spmd

    nc = _get_bass()
    eye = np.eye(TILE_R, dtype=np.float32)
    in_maps = [
        {
            "data": np.ascontiguousarray(data_f32[c * R:(c + 1) * R]),
            "amat": A,
            "ident": eye,
        }
        for c in range(N_CORES)
    ]
    kw = {}
    if trace:
        kw = dict(trace=True, **(trace_kwargs or {}))
    res = run_bass_kernel_spmd(nc, in_maps, core_ids=list(range(N_CORES)), **kw)
    out = np.concatenate([r["out"] for r in res.results], axis=0)
    return out, res


def kernel(data, angles, indices_in, idx_out, _trace=False, _trace_kwargs=None):
    data = np.asarray(data, dtype=np.float32)
    idx_out_np = np.asarray(idx_out, dtype=np.int64)
    fold_scatter = np.unique(idx_out_np).size == W
    A = _compose_matrix(angles, indices_in, idx_out, fold_scatter)
    y, res = _run_device(data, A, trace=_trace, trace_kwargs=_trace_kwargs)
    if not fold_scatter:
        out = np.zeros_like(y)
        out[:, idx_out_np] = y
        y = out
    if _trace:
        return y, res
    return y



# revision 2
# speedup vs baseline: 1.0195x; 1.0195x over previous
"""Trainium2 Bass kernel for nn_ButterflyModule_71176198029535.

The 16 butterfly layers compose into one dense 256x256 linear map A, with the
gather/scatter permutations folded in on the host.  The device computes
out = data @ A as a memory-bound matmul, data-parallel over batch across 8
NeuronCores.

v2 layout strategy (vs v1's on-device PE transposes):
  - Each core's shard is pre-transposed on the host to x^T [256, 16384] so the
    contraction dim (w) is already on SBUF partitions.  The device runs pure
    accumulating matmuls with the A-chunks stationary and x^T moving (N=512
    per PSUM bank); the output y^T [256, 16384] is un-transposed on the host.
  - I/O is bf16 (fp32 PSUM accumulation).  The butterfly map is orthogonal, so
    bf16 quantization of x and A gives max-rel-to-absmax ~4e-3, well under the
    2e-2 gate, and halves HBM traffic: 8 MiB in + 8 MiB out per core.

Device-side per b-group (2048 batch cols, 1 MiB DMA each way):
  - DMA in x^T tile [128p, (2 w-halves x 2048 b)] bf16
  - 4 b-blocks of 512: for each output half m, 2 accumulating matmuls
    (lhsT = A[h, m] 128x128 stationary, rhs = x^T slice [128, 512]) -> PSUM
  - ACT/DVE copy PSUM->SBUF bf16 (one engine per output half)
  - DMA out y^T tile [128p, (2 n-halves x 2048 b)] bf16
"""

import numpy as np

B = 131072
W = 256
LOG2W = 8
L_IN = 8
L_OUT = 8
N_CORES = 8
R = B // N_CORES          # batch cols per core (of x^T)
NB = 512                  # batch cols per matmul (one PSUM bank of fp32)
GROUP_NB = 2048           # batch cols per DMA group (1 MiB bf16 in, 1 MiB out)
N_GROUPS = R // GROUP_NB
BLOCKS = GROUP_NB // NB

_CACHED = {}


def _strides():
    s_in = [1 << (l % LOG2W) for l in range(L_IN)]
    s_out = [1 << ((L_OUT - 1 - k) % LOG2W) for k in range(L_OUT)]
    return s_in + s_out


def _compose_matrix(angles, indices_in, idx_out, fold_scatter):
    """Return A [W, W] float32 such that out = data @ A (when fold_scatter),
    or A' folding only the gather, with the scatter applied on the host."""
    ang = np.asarray(angles, dtype=np.float64)
    M = np.eye(W, dtype=np.float64)
    for l, s in enumerate(_strides()):
        nblk = W // (2 * s)
        blk = np.repeat(np.arange(nblk), s)
        j = np.tile(np.arange(s), nblk)
        ai = blk * 2 * s + j
        bi = blk * 2 * s + s + j
        c = np.cos(ang[l])[:, None]
        sn = np.sin(ang[l])[:, None]
        Ma = M[ai, :].copy()
        Mb = M[bi, :].copy()
        M[ai, :] = c * Ma - sn * Mb
        M[bi, :] = sn * Ma + c * Mb
    # butterfly(z) rows = z @ M.T ; gathered input z = data @ G
    G = np.zeros((W, W), dtype=np.float64)
    G[np.asarray(indices_in, dtype=np.int64), np.arange(W)] = 1.0
    A = G @ M.T
    if fold_scatter:
        S = np.zeros((W, W), dtype=np.float64)
        S[np.arange(W), np.asarray(idx_out, dtype=np.int64)] = 1.0
        A = A @ S
    return np.ascontiguousarray(A, dtype=np.float32)


def _build_bass(repeat=1):
    from contextlib import ExitStack, nullcontext

    import concourse.tile as tile
    from concourse import bacc, mybir

    f32 = mybir.dt.float32
    bf16 = mybir.dt.bfloat16

    nc = bacc.Bacc(trn_type="TRN2", debug=False, num_devices=N_CORES)
    xt = nc.dram_tensor("xt", [W, R], bf16, kind="ExternalInput").ap()
    amat = nc.dram_tensor("amat", [W, W], bf16, kind="ExternalInput").ap()
    yt = nc.dram_tensor("yt", [W, R], bf16, kind="ExternalOutput").ap()

    # Group view: partition p carries w-row p and 128+p; free dims are the
    # half index and the 2048-col slice.  Per-partition DMA runs are 4 KiB.
    in_v = xt.rearrange("(h p) (g b) -> g p h b", h=2, g=N_GROUPS)
    out_v = yt.rearrange("(h p) (g b) -> g p h b", h=2, g=N_GROUPS)

    with tile.TileContext(nc) as tc, ExitStack() as ctx:
        const_pool = ctx.enter_context(tc.tile_pool(name="const", bufs=1))
        in_pool = ctx.enter_context(tc.tile_pool(name="xin", bufs=3))
        out_pool = ctx.enter_context(tc.tile_pool(name="yout", bufs=3))
        ps_pool = ctx.enter_context(tc.tile_pool(name="ps", bufs=8, space="PSUM"))

        # Funnel PE inputs through DVE so each PE op waits on one sem.
        a_sb = []
        for h in range(2):
            a_stage = const_pool.tile([128, W], bf16, tag=f"astage{h}")
            nc.sync.dma_start(a_stage[:], amat[h * 128:(h + 1) * 128, :])
            a_h = const_pool.tile([128, W], bf16, tag=f"a{h}")
            nc.vector.tensor_copy(a_h[:], a_stage[:])
            a_sb.append(a_h)

        # Warm-up PE op consumes the DVE sems for a_sb so steady-state
        # matmuls carry at most one fresh sync wait (DMA or PSUM-free).
        ps_warm = ps_pool.tile([128, NB], f32, tag="ps")
        nc.tensor.matmul(ps_warm[:, 0:W], a_sb[0][:, 0:128], a_sb[1][:], )

        rep_cm = tc.For_i(0, repeat, 1) if repeat > 1 else nullcontext()
        with rep_cm:
            for g in range(N_GROUPS):
                xin = in_pool.tile([128, 2, GROUP_NB], bf16, tag="xin")
                nc.sync.dma_start(xin[:], in_v[g])
                yout = out_pool.tile([128, 2, GROUP_NB], bf16, tag="yout")
                for j in range(BLOCKS):
                    for m in range(2):
                        ps = ps_pool.tile([128, NB], f32, tag="ps")
                        for h in range(2):
                            nc.tensor.matmul(
                                ps[:],
                                a_sb[h][:, m * 128:(m + 1) * 128],
                                xin[:, h, j * NB:(j + 1) * NB],
                                start=(h == 0),
                                stop=(h == 1),
                            )
                        dst = yout[:, m, j * NB:(j + 1) * NB]
                        if m == 0:
                            nc.scalar.copy(dst, ps[:])
                        else:
                            nc.vector.tensor_copy(dst, ps[:])
                nc.sync.dma_start(out_v[g], yout[:])

    nc.compile()
    return nc


def _get_bass(repeat=1):
    key = ("nc", repeat)
    if key not in _CACHED:
        _CACHED[key] = _build_bass(repeat)
    return _CACHED[key]


def _host_prep(data, angles, indices_in, idx_out):
    """Host-side prep: fold gather/scatter into A, quantize to bf16, and lay
    each core's shard out transposed so the device needs no transposes."""
    import ml_dtypes

    data = np.asarray(data, dtype=np.float32)
    idx_out_np = np.asarray(idx_out, dtype=np.int64)
    fold_scatter = np.unique(idx_out_np).size == W
    A = _compose_matrix(angles, indices_in, idx_out, fold_scatter)
    Ab = A.astype(ml_dtypes.bfloat16)
    xb = data.astype(ml_dtypes.bfloat16)
    in_maps = [
        {"xt": np.ascontiguousarray(xb[c * R:(c + 1) * R].T), "amat": Ab}
        for c in range(N_CORES)
    ]
    return in_maps, fold_scatter, idx_out_np


def kernel(data, angles, indices_in, idx_out, _trace=False, _trace_kwargs=None):
    from concourse.bass_utils import run_bass_kernel_spmd

    in_maps, fold_scatter, idx_out_np = _host_prep(
        data, angles, indices_in, idx_out)
    nc = _get_bass()
    kw = {}
    if _trace:
        kw = dict(trace=True, **(_trace_kwargs or {}))
    res = run_bass_kernel_spmd(nc, in_maps, core_ids=list(range(N_CORES)), **kw)
    out = np.empty((B, W), dtype=np.float32)
    for c in range(N_CORES):
        out[c * R:(c + 1) * R] = np.asarray(res.results[c]["yt"]).T
    if not fold_scatter:
        y = np.zeros_like(out)
        y[:, idx_out_np] = out
        out = y
    if _trace:
        return out, res
    return out
